# revision 11
# baseline (speedup 1.0000x reference)
"""Single-head attention (B=4, S=4096, D=1024, DK=128) on 8 TRN2 NeuronCores.

Sharding: core c handles batch b=c//2, query-half h=c%2 (2048 q rows).
K/V are pair-split: each core projects KT/VT for its own 2048 rows; the
other half arrives via a masked ReduceScatter within pairs
[[0,1],[2,3],[4,5],[6,7]]: each core writes its half into the peer's slot
of a [2, ...] bounce buffer and zeros its own slot (0/1 mask delivered as
a per-core input, x+0 == x bitwise), so RS(add) hands every core exactly
its peer's half with a rank-independent graph. Per-core key order is
[own, peer], which is softmax/AV permutation-invariant.

Engine/layout choices (contraction dims must land on SBUF partitions):
  1. x chunks -> PE transpose -> XT [d, s] (f32, rounded to f32r on the
     PSUM->SBUF drain; f32r = 11-bit-mantissa matmul mode, 4x faster than
     f32, end-to-end rel-err ~0.9e-2 vs the 2e-2 gate).
  2. QT[dk,s] = Wq^T XT, KT[dk,s], VT[dk,s] (f32r matmuls, N=512);
     V[s,dk] via PE transpose of VT (bf16).
  3. Flash per 128-q block over 4 key chunks of 1024 (own chunks first so
     they overlap the collective): S = QT_blk^T KT in PSUM (f32r), online
     row-max on DVE, exp+row-sum on ACT (PSUM->SBUF bf16 P), PE-transpose
     P -> PT, AV accumulates Z[q,dk] in PSUM with per-chunk rescale,
     final 1/l on the Z drain.
"""

import numpy as np

import concourse.bass as bass
import concourse.tile as tile
from concourse import bacc, mybir
from concourse.bass_utils import run_bass_kernel_spmd
from concourse.masks import make_identity

B, S, D, DK = 4, 4096, 1024, 128
SH = S // 2          # q rows / own keys per core
NCORES = 8
SCALE = 1.0 / float(np.sqrt(np.float32(DK)))

F32 = mybir.dt.float32
F32R = mybir.dt.float32r
BF16 = mybir.dt.bfloat16

N_DTILE = D // 128            # 8 contraction tiles for projections
S_CHUNK = 512                 # s-chunk for XT/projection phase
N_SCHUNK_OWN = SH // S_CHUNK  # 4
K_CHUNK = 1024                # key chunk in flash phase
N_KCHUNK = S // K_CHUNK       # 4 (2 own + 2 peer)
N_QBLK = SH // 128            # 16

SPLIT_KV = True


def build_bass():
    nc = bacc.Bacc("TRN2", target_bir_lowering=False, debug=False,
                   num_devices=NCORES)
    xq = nc.dram_tensor("xq", [SH, D], F32, kind="ExternalInput").ap()
    xo = (None if SPLIT_KV else
          nc.dram_tensor("xo", [SH, D], F32, kind="ExternalInput").ap())
    wq = nc.dram_tensor("Wq", [D, DK], F32, kind="ExternalInput").ap()
    wk = nc.dram_tensor("Wk", [D, DK], F32, kind="ExternalInput").ap()
    wv = nc.dram_tensor("Wv", [D, DK], F32, kind="ExternalInput").ap()
    if SPLIT_KV:
        pmask = nc.dram_tensor("pmask", [2], F32, kind="ExternalInput").ap()
    z = nc.dram_tensor("z", [SH, DK], F32, kind="ExternalOutput").ap()

    groups = [[0, 1], [2, 3], [4, 5], [6, 7]]

    with tile.TileContext(nc) as tc:
        with (
            tc.tile_pool(name="singles", bufs=1) as singles,
            tc.tile_pool(name="resident", bufs=1) as resident,
            tc.tile_pool(name="dram", bufs=1, space="DRAM") as dram,
        ):
            ident32 = singles.tile([128, 128], F32)
            make_identity(nc, ident32[:])
            identbf = singles.tile([128, 128], BF16)
            make_identity(nc, identbf[:])

            # weights: DRAM [D, DK] -> SBUF [128, N_DTILE, DK] f32r
            w_r = {}
            for name, w in (("wq", wq), ("wk", wk), ("wv", wv)):
                wst = singles.tile([128, N_DTILE, DK], F32, tag=f"{name}_st")
                nc.sync.dma_start(wst[:], w.rearrange("(j p) k -> p j k", p=128))
                wrt = singles.tile([128, N_DTILE, DK], F32R, tag=f"{name}_r")
                nc.any.tensor_copy(wrt[:], wst[:])
                w_r[name] = wrt

            QT = resident.tile([128, SH], F32R)       # [dk, own q rows]
            KTo = resident.tile([128, SH], F32R)      # [dk, own keys]
            VTo = resident.tile([128, SH], BF16)      # [dk, own keys]
            V = resident.tile([128, S // 128, DK], BF16)  # [keys-part, t, dk]
            if SPLIT_KV:
                KTp = resident.tile([128, SH], F32R)  # [dk, peer keys]
                VTp = resident.tile([128, SH], BF16)
                msk = singles.tile([128, 2], F32)
                nc.sync.dma_start(msk[:], pmask.partition_broadcast(128))
                kt_bounce = dram.tile([2, 128, SH], F32)
                kt_out = dram.tile([128, SH], F32)
                vt_bounce = dram.tile([2, 128, SH], BF16)
                vt_out = dram.tile([128, SH], BF16)

            # ---------------- Phase A/B: XT + projections ----------------
            with (
                tc.tile_pool(name="xst", bufs=10) as xst_pool,
                tc.tile_pool(name="xtsb", bufs=2) as xt_pool,
                tc.tile_pool(name="bnc", bufs=2) as bnc_pool,
                tc.tile_pool(name="xtps", bufs=2,
                             space=bass.MemorySpace.PSUM) as xt_ps,
                tc.tile_pool(name="projps", bufs=1,
                             space=bass.MemorySpace.PSUM) as proj_ps,
            ):
                n_chunks = N_SCHUNK_OWN if SPLIT_KV else S // S_CHUNK
                for ci in range(n_chunks):
                    src = xq if ci < N_SCHUNK_OWN else xo
                    off = (ci % N_SCHUNK_OWN) * S_CHUNK

                    xts = []
                    for t in range(S_CHUNK // 128):
                        xt_t = xst_pool.tile([128, D], F32, tag="xst")
                        nc.sync.dma_start(
                            xt_t[:], src[off + t * 128: off + (t + 1) * 128, :])
                        xts.append(xt_t)

                    # transpose to XT chunk [128(d), N_DTILE, S_CHUNK] f32r
                    xt_chunk = xt_pool.tile([128, N_DTILE, S_CHUNK], F32R)
                    for jd2 in range(N_DTILE // 2):
                        ps = xt_ps.tile([128, 2, S_CHUNK], F32, tag="xtps")
                        for j1 in range(2):
                            for t in range(S_CHUNK // 128):
                                nc.tensor.transpose(
                                    ps[:, j1, t * 128:(t + 1) * 128],
                                    xts[t][:, (jd2 * 2 + j1) * 128:
                                           (jd2 * 2 + j1 + 1) * 128],
                                    ident32[:])
                        nc.any.tensor_copy(
                            xt_chunk[:, jd2 * 2:(jd2 + 1) * 2, :], ps[:])

                    # projections for this chunk
                    def do_proj(wkey, dst, dst_off, tag):
                        pps = proj_ps.tile([128, S_CHUNK], F32, tag=tag)
                        for jd in range(N_DTILE):
                            nc.tensor.matmul(
                                pps[:], w_r[wkey][:, jd, :], xt_chunk[:, jd, :],
                                start=(jd == 0), stop=(jd == N_DTILE - 1))
                        nc.any.tensor_copy(
                            dst[:, dst_off:dst_off + S_CHUNK], pps[:])

                    do_proj("wk", KTo if ci < N_SCHUNK_OWN else KTp,
                            off, "k")
                    do_proj("wv", VTo if ci < N_SCHUNK_OWN else VTp,
                            off, "v")
                    if ci < N_SCHUNK_OWN:
                        do_proj("wq", QT, ci * S_CHUNK, "q")

                if SPLIT_KV:
                    # masked bounce: slot r gets KT_own * msk[r]
                    # (msk[h]=0, msk[1-h]=1) then ReduceScatter(add) within
                    # the pair delivers the peer's half.
                    for r in range(2):
                        kb = bnc_pool.tile([128, SH], F32, tag="kb")
                        nc.vector.tensor_scalar_mul(
                            kb[:], KTo[:].bitcast(F32), msk[:, r:r + 1])
                        nc.sync.dma_start(kt_bounce[r], kb[:])
                        vb = bnc_pool.tile([128, SH], BF16, tag="vb")
                        nc.vector.tensor_scalar_mul(
                            vb[:], VTo[:], msk[:, r:r + 1])
                        nc.sync.dma_start(vt_bounce[r], vb[:])
                    nc.gpsimd.collective_compute(
                        "ReduceScatter", mybir.AluOpType.add,
                        replica_groups=groups,
                        ins=[kt_bounce[:].opt()], outs=[kt_out[:].opt()])
                    nc.gpsimd.collective_compute(
                        "ReduceScatter", mybir.AluOpType.add,
                        replica_groups=groups,
                        ins=[vt_bounce[:].opt()], outs=[vt_out[:].opt()])
                    nc.sync.dma_start(KTp[:], kt_out[:].bitcast(F32R))
                    nc.sync.dma_start(VTp[:], vt_out[:])

            # V tiles: transpose VT [dk, s] -> V [s, dk]; own then peer
            with tc.tile_pool(name="vps", bufs=2,
                              space=bass.MemorySpace.PSUM) as v_ps:
                for cg in range(S // 1024):
                    vt_src = VTo if cg < S // 2048 else VTp
                    voff = (cg % (S // 2048)) * 1024
                    vtp = v_ps.tile([128, 1024], BF16)
                    for t in range(8):
                        nc.tensor.transpose(
                            vtp[:, t * 128:(t + 1) * 128],
                            vt_src[:, voff + t * 128: voff + (t + 1) * 128],
                            identbf[:])
                    nc.any.tensor_copy(
                        V[:, cg * 8:(cg + 1) * 8, :],
                        vtp[:].rearrange("p (t k) -> p t k", k=DK))

            # ------- Phase C: flash attention, two half-phases -------
            # Each (q-block, key-half) computes partial z/m/l independently;
            # own-half work overlaps the collective; a batched combine
            # merges the halves at the end.
            NCH = N_KCHUNK // 2  # chunks per half
            zpart = [resident.tile([128, N_QBLK, DK], F32, tag=f"zp{h}",
                                   name=f"zp{h}")
                     for h in range(2)]
            mpart = [resident.tile([128, N_QBLK], F32, tag=f"mp{h}",
                                   name=f"mp{h}")
                     for h in range(2)]
            lpart = [resident.tile([128, N_QBLK], F32, tag=f"lp{h}",
                                   name=f"lp{h}")
                     for h in range(2)]
            with (
                tc.tile_pool(name="psb", bufs=3) as p_pool,
                tc.tile_pool(name="ptsb", bufs=3) as pt_pool,
                tc.tile_pool(name="stats", bufs=12) as stats,
                tc.tile_pool(name="sps", bufs=2,
                             space=bass.MemorySpace.PSUM) as s_ps,
                tc.tile_pool(name="ptps", bufs=2,
                             space=bass.MemorySpace.PSUM) as pt_ps,
                tc.tile_pool(name="zps", bufs=2,
                             space=bass.MemorySpace.PSUM) as z_ps_pool,
            ):
                for half in range(2):
                    kt = KTo if half == 0 else KTp
                    for qb in range(N_QBLK):
                        qsl = QT[:, qb * 128:(qb + 1) * 128]
                        z_acc = z_ps_pool.tile([128, DK], F32, tag="z")
                        m_run = None
                        for j in range(NCH):
                            sps = s_ps.tile([128, K_CHUNK], F32, tag="sps")
                            for hf in range(K_CHUNK // 512):
                                nc.tensor.matmul(
                                    sps[:, hf * 512:(hf + 1) * 512],
                                    qsl,
                                    kt[:, j * K_CHUNK + hf * 512:
                                       j * K_CHUNK + (hf + 1) * 512],
                                    start=True, stop=True)

                            m_j = stats.tile([128, 1], F32, tag="mj")
                            nc.vector.reduce_max(
                                out=m_j[:], in_=sps[:],
                                axis=mybir.AxisListType.X)

                            if j == 0:
                                m_new = m_j
                            else:
                                m_new = stats.tile([128, 1], F32, tag="mnew")
                                nc.vector.tensor_max(
                                    m_new[:], m_run[:], m_j[:])
                                diff = stats.tile([128, 1], F32, tag="diff")
                                nc.vector.tensor_sub(
                                    diff[:], m_run[:], m_new[:])
                                corr = stats.tile([128, 1], F32, tag="corr")
                                nc.scalar.activation(
                                    out=corr[:], in_=diff[:],
                                    func=mybir.ActivationFunctionType.Exp,
                                    scale=SCALE)
                                nc.vector.tensor_scalar_mul(
                                    z_acc[:], z_acc[:], corr[:])

                            neg = stats.tile([128, 1], F32, tag="neg")
                            nc.vector.tensor_scalar_mul(
                                neg[:], m_new[:], -SCALE)

                            p_sb = p_pool.tile([128, K_CHUNK], BF16)
                            l_j = stats.tile([128, 1], F32, tag="lj")
                            nc.scalar.activation(
                                out=p_sb[:], in_=sps[:],
                                func=mybir.ActivationFunctionType.Exp,
                                bias=neg[:], scale=SCALE, accum_out=l_j[:])

                            if j == 0:
                                nc.vector.tensor_copy(
                                    lpart[half][:, qb:qb + 1], l_j[:])
                            else:
                                nc.vector.tensor_scalar(
                                    out=lpart[half][:, qb:qb + 1],
                                    in0=lpart[half][:, qb:qb + 1],
                                    scalar1=corr[:], scalar2=l_j[:],
                                    op0=mybir.AluOpType.mult,
                                    op1=mybir.AluOpType.add)

                            # transpose P chunk -> PT [keys, q]
                            pt_sb = pt_pool.tile(
                                [128, K_CHUNK // 128, 128], BF16)
                            ptp = pt_ps.tile([128, K_CHUNK], BF16)
                            for t in range(K_CHUNK // 128):
                                nc.tensor.transpose(
                                    ptp[:, t * 128:(t + 1) * 128],
                                    p_sb[:, t * 128:(t + 1) * 128],
                                    identbf[:])
                            nc.any.tensor_copy(
                                pt_sb[:],
                                ptp[:].rearrange("p (t k) -> p t k", k=128))

                            # AV accumulate within the half
                            for t in range(K_CHUNK // 128):
                                nc.tensor.matmul(
                                    z_acc[:],
                                    pt_sb[:, t, :],
                                    V[:, (half * NCH + j) * (K_CHUNK // 128)
                                      + t, :],
                                    start=(j == 0 and t == 0),
                                    stop=(j == NCH - 1 and
                                          t == K_CHUNK // 128 - 1),
                                    skip_group_check=True)

                            m_run = m_new

                        nc.vector.tensor_copy(
                            mpart[half][:, qb:qb + 1], m_run[:])
                        nc.any.tensor_copy(
                            zpart[half][:, qb, :], z_acc[:])

                # ---- combine halves (batched stats, gpsimd z-merge) ----
                mm = stats.tile([128, N_QBLK], F32, tag="mm")
                nc.vector.tensor_max(mm[:], mpart[0][:], mpart[1][:])
                cfac = []
                for h in range(2):
                    d = stats.tile([128, N_QBLK], F32, tag=f"d{h}")
                    nc.vector.tensor_sub(d[:], mpart[h][:], mm[:])
                    c = stats.tile([128, N_QBLK], F32, tag=f"c{h}")
                    nc.scalar.activation(
                        out=c[:], in_=d[:],
                        func=mybir.ActivationFunctionType.Exp, scale=SCALE)
                    cfac.append(c)
                lw = []
                for h in range(2):
                    w = stats.tile([128, N_QBLK], F32, tag=f"lw{h}")
                    nc.vector.tensor_mul(w[:], lpart[h][:], cfac[h][:])
                    lw.append(w)
                ltot = stats.tile([128, N_QBLK], F32, tag="ltot")
                nc.vector.tensor_add(ltot[:], lw[0][:], lw[1][:])
                rinv = stats.tile([128, N_QBLK], F32, tag="rinv")
                nc.vector.reciprocal(rinv[:], ltot[:])
                cw = []
                for h in range(2):
                    w = stats.tile([128, N_QBLK], F32, tag=f"cw{h}")
                    nc.vector.tensor_mul(w[:], cfac[h][:], rinv[:])
                    cw.append(w)

                z_all = resident.tile([128, N_QBLK, DK], F32, tag="zall")
                for qb in range(N_QBLK):
                    t0 = stats.tile([128, DK], F32, tag="zt0")
                    nc.gpsimd.tensor_scalar_mul(
                        t0[:], zpart[0][:, qb, :], cw[0][:, qb:qb + 1])
                    t1 = stats.tile([128, DK], F32, tag="zt1")
                    nc.gpsimd.tensor_scalar_mul(
                        t1[:], zpart[1][:, qb, :], cw[1][:, qb:qb + 1])
                    nc.gpsimd.tensor_add(z_all[:, qb, :], t0[:], t1[:])
                nc.sync.dma_start(
                    z.rearrange("(t p) k -> p t k", p=128), z_all[:])

    nc.compile()
    return nc


_NC_CACHE = None


def _get_nc():
    global _NC_CACHE
    if _NC_CACHE is None:
        _NC_CACHE = build_bass()
    return _NC_CACHE


def make_in_maps(x, Wq, Wk, Wv):
    x = np.ascontiguousarray(np.asarray(x, dtype=np.float32))
    Wq = np.ascontiguousarray(np.asarray(Wq, dtype=np.float32))
    Wk = np.ascontiguousarray(np.asarray(Wk, dtype=np.float32))
    Wv = np.ascontiguousarray(np.asarray(Wv, dtype=np.float32))
    in_maps = []
    for c in range(NCORES):
        b, h = divmod(c, 2)
        m = {
            "xq": np.ascontiguousarray(x[b, h * SH:(h + 1) * SH]),
            "Wq": Wq, "Wk": Wk, "Wv": Wv,
        }
        if SPLIT_KV:
            mask = np.zeros(2, np.float32)
            mask[1 - h] = 1.0
            m["pmask"] = mask
        else:
            m["xo"] = np.ascontiguousarray(x[b, (1 - h) * SH:(2 - h) * SH])
        in_maps.append(m)
    return in_maps


def run(x, Wq, Wk, Wv, trace=False, **kwargs):
    nc = _get_nc()
    res = run_bass_kernel_spmd(nc, make_in_maps(x, Wq, Wk, Wv),
                               core_ids=list(range(NCORES)), trace=trace,
                               **kwargs)
    zfull = np.empty((B, S, DK), np.float32)
    for c in range(NCORES):
        b, h = divmod(c, 2)
        zfull[b, h * SH:(h + 1) * SH] = res.results[c]["z"]
    return zfull, res


def kernel(x, Wq, Wk, Wv):
    zfull, _ = run(x, Wq, Wk, Wv)
    return zfull


if __name__ == "__main__":
    rng = np.random.default_rng(0)
    x = rng.standard_normal((B, S, D), dtype=np.float32)
    Wq_ = rng.standard_normal((D, DK), dtype=np.float32)
    Wk_ = rng.standard_normal((D, DK), dtype=np.float32)
    Wv_ = rng.standard_normal((D, DK), dtype=np.float32)
    zk = kernel(x, Wq_, Wk_, Wv_)
    print("kernel output", zk.shape, zk.dtype)


# revision 13
# speedup vs baseline: 1.2088x; 1.2088x over previous
"""Single-head attention (B=4, S=4096, D=1024, DK=128) on 8 TRN2 NeuronCores.

Sharding: core c handles batch b=c//2, query-half h=c%2 (2048 q rows).
K/V are pair-split: each core projects KT/VT for its own 2048 rows; the
other half arrives via a masked ReduceScatter within pairs
[[0,1],[2,3],[4,5],[6,7]]: each core writes its half into the peer's slot
of a [2, ...] bounce buffer and zeros its own slot (0/1 mask delivered as
a per-core input, x+0 == x bitwise), so RS(add) hands every core exactly
its peer's half with a rank-independent graph. Per-core key order is
[own, peer], which is softmax/AV permutation-invariant.

Engine/layout choices (contraction dims must land on SBUF partitions):
  1. x chunks -> PE transpose -> XT [d, s] (f32, rounded to f32r on the
     PSUM->SBUF drain; f32r = 11-bit-mantissa matmul mode, 4x faster than
     f32, end-to-end rel-err ~0.9e-2 vs the 2e-2 gate).
  2. QT[dk,s] = Wq^T XT, KT[dk,s], VT[dk,s] (f32r matmuls, N=512);
     V[s,dk] via PE transpose of VT (bf16).
  3. Flash per 128-q block over 4 key chunks of 1024 (own chunks first so
     they overlap the collective): S = QT_blk^T KT in PSUM (f32r), online
     row-max on DVE, exp+row-sum on ACT (PSUM->SBUF bf16 P), PE-transpose
     P -> PT, AV accumulates Z[q,dk] in PSUM with per-chunk rescale,
     final 1/l on the Z drain.
"""

import numpy as np

import concourse.bass as bass
import concourse.tile as tile
from concourse import bacc, mybir
from concourse.bass_utils import run_bass_kernel_spmd
from concourse.masks import make_identity

B, S, D, DK = 4, 4096, 1024, 128
SH = S // 2          # q rows / own keys per core
NCORES = 8
SCALE = 1.0 / float(np.sqrt(np.float32(DK)))

F32 = mybir.dt.float32
F32R = mybir.dt.float32r
BF16 = mybir.dt.bfloat16

N_DTILE = D // 128            # 8 contraction tiles for projections
S_CHUNK = 512                 # s-chunk for XT/projection phase
N_SCHUNK_OWN = SH // S_CHUNK  # 4
K_CHUNK = 1024                # key chunk in flash phase
N_KCHUNK = S // K_CHUNK       # 4 (2 own + 2 peer)
N_QBLK = SH // 128            # 16

SPLIT_KV = True


def build_bass():
    nc = bacc.Bacc("TRN2", target_bir_lowering=False, debug=False,
                   num_devices=NCORES)
    xq = nc.dram_tensor("xq", [SH, D], F32, kind="ExternalInput").ap()
    xo = (None if SPLIT_KV else
          nc.dram_tensor("xo", [SH, D], F32, kind="ExternalInput").ap())
    wq = nc.dram_tensor("Wq", [D, DK], F32, kind="ExternalInput").ap()
    wk = nc.dram_tensor("Wk", [D, DK], F32, kind="ExternalInput").ap()
    wv = nc.dram_tensor("Wv", [D, DK], F32, kind="ExternalInput").ap()
    if SPLIT_KV:
        pmask = nc.dram_tensor("pmask", [2], F32, kind="ExternalInput").ap()
    z = nc.dram_tensor("z", [SH, DK], F32, kind="ExternalOutput").ap()

    groups = [[0, 1], [2, 3], [4, 5], [6, 7]]

    with tile.TileContext(nc) as tc:
        with (
            tc.tile_pool(name="singles", bufs=1) as singles,
            tc.tile_pool(name="resident", bufs=1) as resident,
            tc.tile_pool(name="dram", bufs=1, space="DRAM") as dram,
        ):
            ident32 = singles.tile([128, 128], F32)
            make_identity(nc, ident32[:])
            identbf = singles.tile([128, 128], BF16)
            make_identity(nc, identbf[:])

            # weights: DRAM [D, DK] -> SBUF [128, N_DTILE, DK] f32r
            w_r = {}
            for name, w in (("wq", wq), ("wk", wk), ("wv", wv)):
                wst = singles.tile([128, N_DTILE, DK], F32, tag=f"{name}_st")
                nc.sync.dma_start(wst[:], w.rearrange("(j p) k -> p j k", p=128))
                wrt = singles.tile([128, N_DTILE, DK], F32R, tag=f"{name}_r")
                nc.any.tensor_copy(wrt[:], wst[:])
                w_r[name] = wrt

            QT = resident.tile([128, SH], F32R)       # [dk, own q rows]
            KTo = resident.tile([128, SH], F32R)      # [dk, own keys]
            VTo = resident.tile([128, SH], BF16)      # [dk, own keys]
            Vo = resident.tile([128, SH // 128, DK], BF16)
            Vp = resident.tile([128, SH // 128, DK], BF16)
            if SPLIT_KV:
                KTp = resident.tile([128, SH], F32R)  # [dk, peer keys]
                VTp = resident.tile([128, SH], BF16)
                msk = singles.tile([128, 2], F32)
                nc.sync.dma_start(msk[:], pmask.partition_broadcast(128))
                kt_bounce = dram.tile([2, 128, SH], F32)
                kt_out = dram.tile([128, SH], F32)
                vt_bounce = dram.tile([2, 128, SH], BF16)
                vt_out = dram.tile([128, SH], BF16)

            # ---------------- Phase A/B: XT + projections ----------------
            with (
                tc.tile_pool(name="xst", bufs=10) as xst_pool,
                tc.tile_pool(name="xtsb", bufs=2) as xt_pool,
                tc.tile_pool(name="bnc", bufs=2) as bnc_pool,
                tc.tile_pool(name="xtps", bufs=2,
                             space=bass.MemorySpace.PSUM) as xt_ps,
                tc.tile_pool(name="projps", bufs=1,
                             space=bass.MemorySpace.PSUM) as proj_ps,
            ):
                n_chunks = N_SCHUNK_OWN if SPLIT_KV else S // S_CHUNK
                for ci in range(n_chunks):
                    src = xq if ci < N_SCHUNK_OWN else xo
                    off = (ci % N_SCHUNK_OWN) * S_CHUNK

                    xts = []
                    for t in range(S_CHUNK // 128):
                        xt_t = xst_pool.tile([128, D], F32, tag="xst")
                        nc.sync.dma_start(
                            xt_t[:], src[off + t * 128: off + (t + 1) * 128, :])
                        xts.append(xt_t)

                    # transpose to XT chunk [128(d), N_DTILE, S_CHUNK] f32r
                    xt_chunk = xt_pool.tile([128, N_DTILE, S_CHUNK], F32R)
                    for jd2 in range(N_DTILE // 2):
                        ps = xt_ps.tile([128, 2, S_CHUNK], F32, tag="xtps")
                        for j1 in range(2):
                            for t in range(S_CHUNK // 128):
                                nc.tensor.transpose(
                                    ps[:, j1, t * 128:(t + 1) * 128],
                                    xts[t][:, (jd2 * 2 + j1) * 128:
                                           (jd2 * 2 + j1 + 1) * 128],
                                    ident32[:])
                        nc.any.tensor_copy(
                            xt_chunk[:, jd2 * 2:(jd2 + 1) * 2, :], ps[:])

                    # projections for this chunk
                    def do_proj(wkey, dst, dst_off, tag):
                        pps = proj_ps.tile([128, S_CHUNK], F32, tag=tag)
                        for jd in range(N_DTILE):
                            nc.tensor.matmul(
                                pps[:], w_r[wkey][:, jd, :], xt_chunk[:, jd, :],
                                start=(jd == 0), stop=(jd == N_DTILE - 1))
                        nc.any.tensor_copy(
                            dst[:, dst_off:dst_off + S_CHUNK], pps[:])

                    do_proj("wk", KTo if ci < N_SCHUNK_OWN else KTp,
                            off, "k")
                    do_proj("wv", VTo if ci < N_SCHUNK_OWN else VTp,
                            off, "v")
                    if ci < N_SCHUNK_OWN:
                        do_proj("wq", QT, ci * S_CHUNK, "q")

                if SPLIT_KV:
                    # masked bounce: slot r gets KT_own * msk[r]
                    # (msk[h]=0, msk[1-h]=1) then ReduceScatter(add) within
                    # the pair delivers the peer's half.
                    for r in range(2):
                        kb = bnc_pool.tile([128, SH], F32, tag="kb")
                        nc.vector.tensor_scalar_mul(
                            kb[:], KTo[:].bitcast(F32), msk[:, r:r + 1])
                        nc.sync.dma_start(kt_bounce[r], kb[:])
                        vb = bnc_pool.tile([128, SH], BF16, tag="vb")
                        nc.vector.tensor_scalar_mul(
                            vb[:], VTo[:], msk[:, r:r + 1])
                        nc.sync.dma_start(vt_bounce[r], vb[:])
                    nc.gpsimd.collective_compute(
                        "ReduceScatter", mybir.AluOpType.add,
                        replica_groups=groups,
                        ins=[kt_bounce[:].opt()], outs=[kt_out[:].opt()])
                    nc.gpsimd.collective_compute(
                        "ReduceScatter", mybir.AluOpType.add,
                        replica_groups=groups,
                        ins=[vt_bounce[:].opt()], outs=[vt_out[:].opt()])
                    nc.sync.dma_start(KTp[:], kt_out[:].bitcast(F32R))
                    nc.sync.dma_start(VTp[:], vt_out[:])

            # V tiles: transpose VT [dk, s] -> V [s, dk]; own then peer
            with tc.tile_pool(name="vps", bufs=2,
                              space=bass.MemorySpace.PSUM) as v_ps:
                for cg in range(S // 1024):
                    own = cg < S // 2048
                    vt_src = VTo if own else VTp
                    vdst = Vo if own else Vp
                    voff = (cg % (S // 2048)) * 1024
                    vtp = v_ps.tile([128, 1024], BF16)
                    for t in range(8):
                        nc.tensor.transpose(
                            vtp[:, t * 128:(t + 1) * 128],
                            vt_src[:, voff + t * 128: voff + (t + 1) * 128],
                            identbf[:])
                    nc.any.tensor_copy(
                        vdst[:, (cg % (S // 2048)) * 8:
                             (cg % (S // 2048) + 1) * 8, :],
                        vtp[:].rearrange("p (t k) -> p t k", k=DK))

            # ------- Phase C: flash attention, two half-phases -------
            # Each (q-block, key-half) computes partial z/m/l independently;
            # own-half work overlaps the collective; a batched combine
            # merges the halves at the end.
            NCH = N_KCHUNK // 2  # chunks per half
            zpart = [resident.tile([128, N_QBLK, DK], F32, tag=f"zp{h}",
                                   name=f"zp{h}")
                     for h in range(2)]
            mpart = [resident.tile([128, N_QBLK], F32, tag=f"mp{h}",
                                   name=f"mp{h}")
                     for h in range(2)]
            lpart = [resident.tile([128, N_QBLK], F32, tag=f"lp{h}",
                                   name=f"lp{h}")
                     for h in range(2)]
            with (
                tc.tile_pool(name="psb", bufs=3) as p_pool,
                tc.tile_pool(name="ptsb", bufs=3) as pt_pool,
                tc.tile_pool(name="stats", bufs=12) as stats,
                tc.tile_pool(name="sps", bufs=2,
                             space=bass.MemorySpace.PSUM) as s_ps,
                tc.tile_pool(name="ptps", bufs=2,
                             space=bass.MemorySpace.PSUM) as pt_ps,
                tc.tile_pool(name="zps", bufs=2,
                             space=bass.MemorySpace.PSUM) as z_ps_pool,
            ):
                for half in range(2):
                    kt = KTo if half == 0 else KTp
                    for qb in range(N_QBLK):
                        qsl = QT[:, qb * 128:(qb + 1) * 128]
                        z_acc = z_ps_pool.tile([128, DK], F32, tag="z")
                        m_run = None
                        for j in range(NCH):
                            sps = s_ps.tile([128, K_CHUNK], F32, tag="sps")
                            for hf in range(K_CHUNK // 512):
                                nc.tensor.matmul(
                                    sps[:, hf * 512:(hf + 1) * 512],
                                    qsl,
                                    kt[:, j * K_CHUNK + hf * 512:
                                       j * K_CHUNK + (hf + 1) * 512],
                                    start=True, stop=True)

                            m_j = stats.tile([128, 1], F32, tag="mj")
                            nc.vector.reduce_max(
                                out=m_j[:], in_=sps[:],
                                axis=mybir.AxisListType.X)

                            if j == 0:
                                m_new = m_j
                            else:
                                m_new = stats.tile([128, 1], F32, tag="mnew")
                                nc.vector.tensor_max(
                                    m_new[:], m_run[:], m_j[:])
                                diff = stats.tile([128, 1], F32, tag="diff")
                                nc.vector.tensor_sub(
                                    diff[:], m_run[:], m_new[:])
                                corr = stats.tile([128, 1], F32, tag="corr")
                                nc.scalar.activation(
                                    out=corr[:], in_=diff[:],
                                    func=mybir.ActivationFunctionType.Exp,
                                    scale=SCALE)
                                nc.vector.tensor_scalar_mul(
                                    z_acc[:], z_acc[:], corr[:])

                            neg = stats.tile([128, 1], F32, tag="neg")
                            nc.vector.tensor_scalar_mul(
                                neg[:], m_new[:], -SCALE)

                            p_sb = p_pool.tile([128, K_CHUNK], BF16)
                            l_j = stats.tile([128, 1], F32, tag="lj")
                            nc.scalar.activation(
                                out=p_sb[:], in_=sps[:],
                                func=mybir.ActivationFunctionType.Exp,
                                bias=neg[:], scale=SCALE, accum_out=l_j[:])

                            if j == 0:
                                l_run = l_j
                            else:
                                l_new = stats.tile([128, 1], F32, tag="lnew")
                                nc.vector.tensor_scalar(
                                    out=l_new[:], in0=l_run[:],
                                    scalar1=corr[:], scalar2=l_j[:],
                                    op0=mybir.AluOpType.mult,
                                    op1=mybir.AluOpType.add)
                                l_run = l_new

                            # transpose P chunk -> PT [keys, q]
                            pt_sb = pt_pool.tile(
                                [128, K_CHUNK // 128, 128], BF16)
                            ptp = pt_ps.tile([128, K_CHUNK], BF16)
                            for t in range(K_CHUNK // 128):
                                nc.tensor.transpose(
                                    ptp[:, t * 128:(t + 1) * 128],
                                    p_sb[:, t * 128:(t + 1) * 128],
                                    identbf[:])
                            nc.any.tensor_copy(
                                pt_sb[:],
                                ptp[:].rearrange("p (t k) -> p t k", k=128))

                            # AV accumulate within the half
                            vsrc = Vo if half == 0 else Vp
                            for t in range(K_CHUNK // 128):
                                nc.tensor.matmul(
                                    z_acc[:],
                                    pt_sb[:, t, :],
                                    vsrc[:, j * (K_CHUNK // 128) + t, :],
                                    start=(j == 0 and t == 0),
                                    stop=(j == NCH - 1 and
                                          t == K_CHUNK // 128 - 1),
                                    skip_group_check=True)

                            m_run = m_new

                        nc.vector.tensor_copy(
                            mpart[half][:, qb:qb + 1], m_run[:])
                        nc.vector.tensor_copy(
                            lpart[half][:, qb:qb + 1], l_run[:])
                        nc.any.tensor_copy(
                            zpart[half][:, qb, :], z_acc[:])

                # ---- combine halves (batched stats, gpsimd z-merge) ----
                mm = stats.tile([128, N_QBLK], F32, tag="mm")
                nc.vector.tensor_max(mm[:], mpart[0][:], mpart[1][:])
                cfac = []
                for h in range(2):
                    d = stats.tile([128, N_QBLK], F32, tag=f"d{h}")
                    nc.vector.tensor_sub(d[:], mpart[h][:], mm[:])
                    c = stats.tile([128, N_QBLK], F32, tag=f"c{h}")
                    nc.scalar.activation(
                        out=c[:], in_=d[:],
                        func=mybir.ActivationFunctionType.Exp, scale=SCALE)
                    cfac.append(c)
                lw = []
                for h in range(2):
                    w = stats.tile([128, N_QBLK], F32, tag=f"lw{h}")
                    nc.vector.tensor_mul(w[:], lpart[h][:], cfac[h][:])
                    lw.append(w)
                ltot = stats.tile([128, N_QBLK], F32, tag="ltot")
                nc.vector.tensor_add(ltot[:], lw[0][:], lw[1][:])
                rinv = stats.tile([128, N_QBLK], F32, tag="rinv")
                nc.vector.reciprocal(rinv[:], ltot[:])
                cw = []
                for h in range(2):
                    w = stats.tile([128, N_QBLK], F32, tag=f"cw{h}")
                    nc.vector.tensor_mul(w[:], cfac[h][:], rinv[:])
                    cw.append(w)

                z_all = resident.tile([128, N_QBLK, DK], F32, tag="zall")
                for qb in range(N_QBLK):
                    t0 = stats.tile([128, DK], F32, tag="zt0")
                    nc.vector.tensor_scalar_mul(
                        t0[:], zpart[0][:, qb, :], cw[0][:, qb:qb + 1])
                    t1 = stats.tile([128, DK], F32, tag="zt1")
                    nc.vector.tensor_scalar_mul(
                        t1[:], zpart[1][:, qb, :], cw[1][:, qb:qb + 1])
                    nc.vector.tensor_add(z_all[:, qb, :], t0[:], t1[:])
                nc.sync.dma_start(
                    z.rearrange("(t p) k -> p t k", p=128), z_all[:])

    nc.compile()
    return nc


_NC_CACHE = None


def _get_nc():
    global _NC_CACHE
    if _NC_CACHE is None:
        _NC_CACHE = build_bass()
    return _NC_CACHE


def make_in_maps(x, Wq, Wk, Wv):
    x = np.ascontiguousarray(np.asarray(x, dtype=np.float32))
    Wq = np.ascontiguousarray(np.asarray(Wq, dtype=np.float32))
    Wk = np.ascontiguousarray(np.asarray(Wk, dtype=np.float32))
    Wv = np.ascontiguousarray(np.asarray(Wv, dtype=np.float32))
    in_maps = []
    for c in range(NCORES):
        b, h = divmod(c, 2)
        m = {
            "xq": np.ascontiguousarray(x[b, h * SH:(h + 1) * SH]),
            "Wq": Wq, "Wk": Wk, "Wv": Wv,
        }
        if SPLIT_KV:
            mask = np.zeros(2, np.float32)
            mask[1 - h] = 1.0
            m["pmask"] = mask
        else:
            m["xo"] = np.ascontiguousarray(x[b, (1 - h) * SH:(2 - h) * SH])
        in_maps.append(m)
    return in_maps


def run(x, Wq, Wk, Wv, trace=False, **kwargs):
    nc = _get_nc()
    res = run_bass_kernel_spmd(nc, make_in_maps(x, Wq, Wk, Wv),
                               core_ids=list(range(NCORES)), trace=trace,
                               **kwargs)
    zfull = np.empty((B, S, DK), np.float32)
    for c in range(NCORES):
        b, h = divmod(c, 2)
        zfull[b, h * SH:(h + 1) * SH] = res.results[c]["z"]
    return zfull, res


def kernel(x, Wq, Wk, Wv):
    zfull, _ = run(x, Wq, Wk, Wv)
    return zfull


if __name__ == "__main__":
    rng = np.random.default_rng(0)
    x = rng.standard_normal((B, S, D), dtype=np.float32)
    Wq_ = rng.standard_normal((D, DK), dtype=np.float32)
    Wk_ = rng.standard_normal((D, DK), dtype=np.float32)
    Wv_ = rng.standard_normal((D, DK), dtype=np.float32)
    zk = kernel(x, Wq_, Wk_, Wv_)
    print("kernel output", zk.shape, zk.dtype)
